# revision 27
# baseline (speedup 1.0000x reference)
"""Trainium2 Bass kernel for nn_DynamicMemoryCell.

Reference computation (per batch element b, N=2048 rows):
    q = x @ Wq.T + bq                  # [N, 128]
    k = h @ Wk.T + bk                  # [N, 128]
    probs = softmax(q @ k.T, axis=-1)  # [N, N]   (output 2)
    u = sigmoid([x, h] @ Wu.T + bu)    # [N, 256]
    r = sigmoid([x, h] @ Wr.T + br)    # [N, 256]
    c = tanh([x, r*h] @ Wc.T + bc)     # [N, 256]
    new_mem = (1-u)*h + u*c            # [N, 256]  (output 1)
(The reference's context_vector / Wv / bv are dead code — not computed here.)

Sharding: data-parallel over batch, one batch element per NeuronCore (B=8).

On-chip layout is feature-major ("transposed"): big tensors live as
[feature (partitions), N (free)] so matmul contractions run over the
partition dim and all biases are per-partition scalars.  The host
pre-transposes inputs / packs weights and transposes new_mem back, so the
device does zero transposes.  Matmul inputs are fp16 (fp16*fp16 products
are exact in fp32 accumulation; input rounding error ~5e-4 relative).
Softmax skips max-subtraction: |scores| <= ~40 so exp() stays in fp32
range, and softmax output is mathematically shift-invariant.

Scheduling: the q/k projections run first, then the 16-query-tile
scores->exp->normalize->store pipeline (memory-bound: 1 MB of probs per
tile).  Gate work is interleaved between score tiles so TensorE/ScalarE
fill the gaps under the DMA-bound probs stream.  Sigmoids are computed as
0.5 + 0.5*tanh(x/2) because tanh lives in the same ScalarE table set as
exp ("exp_and_others") while sigmoid does not — this avoids ~2.7us table
reloads and lets gate activations interleave with the exp stream.  The
q/k bias adds and the 0.5*t+0.5 affines run on VectorE, the new-memory
blend on VectorE, so ScalarE does only tanh/exp.
"""

import numpy as np

B, N = 8, 2048
IN_DIM, MEM_DIM, CTX = 256, 256, 128
NT = N // 512  # 4 free-dim tiles of 512 per [128, N] psum tile
QT = N // 128  # 16 query tiles

_BUILT = None  # cached nc so repeated kernel() calls reuse the program


def _build_nc():
    import concourse.bacc as bacc
    import concourse.tile as tile
    from concourse import mybir

    f16 = mybir.dt.float16
    f32 = mybir.dt.float32
    AF = mybir.ActivationFunctionType
    _MULT = mybir.AluOpType.mult
    _ADD = mybir.AluOpType.add

    # Bacc (not plain Bass): its finalize() runs the lowering passes that
    # split multi-sem waits onto EventSemaphore instructions (TRN2 allows
    # only one sem wait per regular instruction) and insert ACT table loads.
    nc = bacc.Bacc("TRN2", target_bir_lowering=False, debug=False)

    # ---- DRAM parameters (per core) ----
    xT16_d = nc.declare_dram_parameter("xT16", [2, 128, N], f16, isOutput=False)
    hT16_d = nc.declare_dram_parameter("hT16", [2, 128, N], f16, isOutput=False)
    hT32_d = nc.declare_dram_parameter("hT32", [2, 128, N], f32, isOutput=False)
    # weights packed as [128 (contraction-within-chunk), kchunk * out_features]
    wq_d = nc.declare_dram_parameter("wq", [128, 2 * CTX], f16, isOutput=False)
    wk_d = nc.declare_dram_parameter("wk", [128, 2 * CTX], f16, isOutput=False)
    wu_d = nc.declare_dram_parameter("wu", [128, 4 * MEM_DIM], f16, isOutput=False)
    wr_d = nc.declare_dram_parameter("wr", [128, 4 * MEM_DIM], f16, isOutput=False)
    wc_d = nc.declare_dram_parameter("wc", [128, 4 * MEM_DIM], f16, isOutput=False)
    # biases, packed per 128-partition chunk: bq, bk raw; br, bu pre-halved
    # (tanh path: sigmoid(x) = 0.5 + 0.5*tanh(0.5*x + b/2)); bc raw.
    bq_d = nc.declare_dram_parameter("bq", [128, 1], f32, isOutput=False)
    bk_d = nc.declare_dram_parameter("bk", [128, 1], f32, isOutput=False)
    buh_d = nc.declare_dram_parameter("buh", [2, 128, 1], f32, isOutput=False)
    brh_d = nc.declare_dram_parameter("brh", [2, 128, 1], f32, isOutput=False)
    bc_d = nc.declare_dram_parameter("bc", [2, 128, 1], f32, isOutput=False)
    # outputs.  probs leaves the device as fp16: values are normalized
    # softmax outputs in [0,1] (well inside fp16 range) and halving the
    # dominant 16.8 MB/core store is worth the ~5e-4 rounding; the host
    # upcasts to fp32.
    probs_d = nc.declare_dram_parameter("probs", [N, N], f16, isOutput=True)
    nmT_d = nc.declare_dram_parameter("nmT", [2, 128, N], f32, isOutput=True)

    with tile.TileContext(nc) as tc:
        with (
            tc.tile_pool(name="const", bufs=1) as const,
            tc.tile_pool(name="big", bufs=1) as big,
            tc.tile_pool(name="probsp", bufs=5) as probsp,
            tc.tile_pool(name="small", bufs=16) as small,
            tc.tile_pool(name="scr", bufs=2) as scr,
            tc.tile_pool(name="et32p", bufs=3) as et32p,
            tc.tile_pool(name="psum", bufs=2, space="PSUM") as psum,
        ):
            # ---- loads, critical-path first ----
            wq_sb = const.tile([128, 2 * CTX], f16, name="wq_sb")
            wk_sb = const.tile([128, 2 * CTX], f16, name="wk_sb")
            nc.sync.dma_start(out=wq_sb, in_=wq_d[:, :])
            nc.sync.dma_start(out=wk_sb, in_=wk_d[:, :])
            bq_sb = const.tile([128, 1], f32, name="bq_sb")
            bk_sb = const.tile([128, 1], f32, name="bk_sb")
            nc.sync.dma_start(out=bq_sb, in_=bq_d[:, :])
            nc.sync.dma_start(out=bk_sb, in_=bk_d[:, :])

            xt = []
            ht16 = []
            for i in range(2):
                x_i = big.tile([128, N], f16, name=f"xt{i}")
                nc.sync.dma_start(out=x_i, in_=xT16_d[i, :, :])
                xt.append(x_i)
            for i in range(2):
                h_i = big.tile([128, N], f16, name=f"ht16_{i}")
                nc.sync.dma_start(out=h_i, in_=hT16_d[i, :, :])
                ht16.append(h_i)

            wr_sb = const.tile([128, 4 * MEM_DIM], f16, name="wr_sb")
            wu_sb = const.tile([128, 4 * MEM_DIM], f16, name="wu_sb")
            wc_sb = const.tile([128, 4 * MEM_DIM], f16, name="wc_sb")
            nc.sync.dma_start(out=wr_sb, in_=wr_d[:, :])
            nc.sync.dma_start(out=wu_sb, in_=wu_d[:, :])
            nc.sync.dma_start(out=wc_sb, in_=wc_d[:, :])
            brh_sb = const.tile([128, 2], f32, name="brh_sb")
            buh_sb = const.tile([128, 2], f32, name="buh_sb")
            bc_sb = const.tile([128, 2], f32, name="bc_sb")
            for m in range(2):
                nc.sync.dma_start(out=brh_sb[:, m : m + 1], in_=brh_d[m, :, :])
                nc.sync.dma_start(out=buh_sb[:, m : m + 1], in_=buh_d[m, :, :])
                nc.sync.dma_start(out=bc_sb[:, m : m + 1], in_=bc_d[m, :, :])
            ht32 = []
            for i in range(2):
                h32_i = big.tile([128, N], f32, name=f"ht32_{i}")
                nc.sync.dma_start(out=h32_i, in_=hT32_d[i, :, :])
                ht32.append(h32_i)

            combined = xt + ht16  # 4 contraction chunks of the gate matmuls

            # ---- PE warmup ----
            # The PE clock sits at 1.2 GHz until ~3.4us of sustained
            # activity (HAM gate).  Input DMA takes ~6us anyway, so warm the
            # array on a zeroed tile now and the q/k/score matmuls run at
            # 2.4 GHz from the start.
            warm = scr.tile([128, 512], f16, name="warm", tag="warm")
            nc.vector.memset(warm, 0.0)
            wps = psum.tile([128, N], f32, name="wps", tag="ps")
            for _ in range(16):
                nc.tensor.matmul(
                    wps[:, 0:512], lhsT=warm[:, 0:128], rhs=warm,
                    start=True, stop=True,
                )

            # ---- q/k projections (CTX=128 partitions) ----
            # PSUM->SBUF bias+cast: q on VectorE, k on ScalarE Identity —
            # they run in parallel, shortening the path to the first exp.
            q16 = big.tile([128, N], f16, name="q16")
            k16 = big.tile([128, N], f16, name="k16")
            for (w_sb, src, b_sb, dst, via_act) in (
                (wq_sb, xt, bq_sb, q16, False),
                (wk_sb, ht16, bk_sb, k16, True),
            ):
                ps = psum.tile([128, N], f32, name="ps_qk", tag="ps")
                for j in range(NT):
                    js = slice(j * 512, (j + 1) * 512)
                    for k in range(2):
                        nc.tensor.matmul(
                            ps[:, js],
                            lhsT=w_sb[:, k * CTX : (k + 1) * CTX],
                            rhs=src[k][:, js],
                            start=(k == 0),
                            stop=(k == 1),
                        )
                if via_act:
                    nc.scalar.activation(dst, ps, AF.Identity, bias=b_sb)
                else:
                    nc.vector.tensor_scalar_add(dst, ps, b_sb)

            # ---- gate building blocks (emitted interleaved with scores) ----
            def gate_matmuls(w_sb, chunks, m):
                ps = psum.tile([128, N], f32, name="ps_g", tag="ps")
                for j in range(NT):
                    js = slice(j * 512, (j + 1) * 512)
                    for k in range(4):
                        col = k * MEM_DIM + m * 128
                        nc.tensor.matmul(
                            ps[:, js],
                            lhsT=w_sb[:, col : col + 128],
                            rhs=chunks[k][:, js],
                            start=(k == 0),
                            stop=(k == 3),
                        )
                return ps

            r16 = [None, None]
            rh16 = [None, None]
            u16 = [None, None]
            c16 = [None, None]
            p1 = [None, None]  # h*(1-u), precomputed before c is ready

            def r_block(m):
                ps = gate_matmuls(wr_sb, combined, m)
                t = scr.tile([128, N], f16, name="t16", tag="t16")
                nc.scalar.activation(
                    t, ps, AF.Tanh, bias=brh_sb[:, m : m + 1], scale=0.5
                )
                r_m = big.tile([128, N], f16, name=f"r16_{m}")
                nc.vector.tensor_scalar(
                    r_m, t, 0.5, 0.5,
                    op0=_MULT, op1=_ADD,
                )
                rh_m = big.tile([128, N], f16, name=f"rh16_{m}")
                nc.vector.tensor_mul(rh_m, r_m, ht16[m])
                r16[m], rh16[m] = r_m, rh_m

            def u_block(m):
                ps = gate_matmuls(wu_sb, combined, m)
                t = scr.tile([128, N], f16, name="t16", tag="t16")
                nc.scalar.activation(
                    t, ps, AF.Tanh, bias=buh_sb[:, m : m + 1], scale=0.5
                )
                u_m = big.tile([128, N], f16, name=f"u16_{m}")
                nc.vector.tensor_scalar(
                    u_m, t, 0.5, 0.5,
                    op0=_MULT, op1=_ADD,
                )
                u16[m] = u_m
                # 1-u = 0.5 - 0.5*t, then h*(1-u): hides the blend work
                # that doesn't need c, shortening the post-c tail.
                w_m = scr.tile([128, N], f16, name="t16b", tag="t16b")
                nc.vector.tensor_scalar(
                    w_m, t, -0.5, 0.5,
                    op0=_MULT, op1=_ADD,
                )
                p1_m = big.tile([128, N], f32, name=f"p1_{m}")
                nc.vector.tensor_mul(p1_m, w_m, ht32[m])
                p1[m] = p1_m

            def c_block(m):
                ps = gate_matmuls(wc_sb, xt + [rh16[0], rh16[1]], m)
                c_m = big.tile([128, N], f16, name=f"c16_{m}")
                nc.scalar.activation(c_m, ps, AF.Tanh, bias=bc_sb[:, m : m + 1])
                c16[m] = c_m

            def nm_block(m):
                # new_mem = h*(1-u) + u*c, with h*(1-u) precomputed
                t2 = scr.tile([128, N], f16, name="t16c", tag="t16c")
                nc.vector.tensor_mul(t2, u16[m], c16[m])
                nm_m = big.tile([128, N], f32, name=f"nm_{m}")
                nc.vector.tensor_add(nm_m, t2, p1[m])
                nc.sync.dma_start(out=nmT_d[m, :, :], in_=nm_m)

            # ---- one score tile: 128 q rows -> softmax probs -> DRAM ----
            # exp writes fp32 (unnormalized exps overflow fp16), the
            # normalize pass casts to fp16 for the store.
            def score_tile(q):
                ps = psum.tile([128, N], f32, name="ps_s", tag="ps")
                for j in range(NT):
                    js = slice(j * 512, (j + 1) * 512)
                    nc.tensor.matmul(
                        ps[:, js],
                        lhsT=q16[:, q * 128 : (q + 1) * 128],
                        rhs=k16[:, js],
                        start=True,
                        stop=True,
                    )
                et = et32p.tile([128, N], f32, name="et", tag="et")
                ssum = small.tile([128, 1], f32, name="ssum", tag="ssum")
                nc.scalar.activation(et, ps, AF.Exp, accum_out=ssum)
                rsum = small.tile([128, 1], f32, name="rsum", tag="rsum")
                nc.vector.reciprocal(rsum, ssum)
                etp = probsp.tile([128, N], f16, name="etp", tag="etp")
                nc.vector.tensor_scalar_mul(etp, et, rsum)
                nc.sync.dma_start(out=probs_d[q * 128 : (q + 1) * 128, :], in_=etp)

            # ---- interleaved schedule: gate work shares the two PSUM
            # slots with the score tiles, filling TensorE/ScalarE gaps in
            # the DMA-bound probs pipeline ----
            gate_work = {
                1: lambda: r_block(0),
                3: lambda: r_block(1),
                6: lambda: u_block(0),
                8: lambda: u_block(1),
                10: lambda: c_block(0),
                12: lambda: c_block(1),
                13: lambda: nm_block(0),
                14: lambda: nm_block(1),
            }
            for q in range(QT):
                score_tile(q)
                w = gate_work.get(q)
                if w is not None:
                    w()

    nc.finalize()
    return nc


def _pack_qk(w):
    # Wq/Wk [CTX=128, D=256] -> lhsT chunks [128 (d within chunk), 2*128]
    return np.ascontiguousarray(
        w.T.reshape(2, 128, CTX).transpose(1, 0, 2).reshape(128, 2 * CTX)
    ).astype(np.float16)


def _pack_gate(w):
    # Wu/Wr/Wc [M=256, D=512] -> lhsT chunks [128 (d within chunk), 4*256]
    return np.ascontiguousarray(
        w.T.reshape(4, 128, MEM_DIM).transpose(1, 0, 2).reshape(128, 4 * MEM_DIM)
    ).astype(np.float16)


def _in_maps(inputs):
    x = np.asarray(inputs["input_tensor"], np.float32)   # [B, N, 256]
    h = np.asarray(inputs["previous_memory"], np.float32)
    shared = {
        "wq": _pack_qk(np.asarray(inputs["Wq"], np.float32)),
        "wk": _pack_qk(np.asarray(inputs["Wk"], np.float32)),
        "wu": _pack_gate(np.asarray(inputs["Wu"], np.float32)),
        "wr": _pack_gate(np.asarray(inputs["Wr"], np.float32)),
        "wc": _pack_gate(np.asarray(inputs["Wc"], np.float32)),
        "bq": np.asarray(inputs["bq"], np.float32).reshape(128, 1),
        "bk": np.asarray(inputs["bk"], np.float32).reshape(128, 1),
        "buh": np.asarray(inputs["bu"], np.float32).reshape(2, 128, 1) * 0.5,
        "brh": np.asarray(inputs["br"], np.float32).reshape(2, 128, 1) * 0.5,
        "bc": np.asarray(inputs["bc"], np.float32).reshape(2, 128, 1),
    }
    in_maps = []
    for b in range(B):
        xT = np.ascontiguousarray(x[b].T)  # [256, N]
        hT = np.ascontiguousarray(h[b].T)
        m = dict(shared)
        m["xT16"] = xT.reshape(2, 128, N).astype(np.float16)
        m["hT16"] = hT.reshape(2, 128, N).astype(np.float16)
        m["hT32"] = hT.reshape(2, 128, N)
        in_maps.append(m)
    return in_maps


def kernel(**inputs):
    from concourse.bass_utils import run_bass_kernel_spmd

    global _BUILT
    if _BUILT is None:
        _BUILT = _build_nc()
    nc = _BUILT

    res = run_bass_kernel_spmd(nc, _in_maps(inputs), list(range(B))).results

    probs = np.stack([res[b]["probs"] for b in range(B)]).astype(np.float32)
    new_mem = np.stack(
        [res[b]["nmT"].reshape(MEM_DIM, N).T for b in range(B)]
    )
    return np.ascontiguousarray(new_mem), probs


# revision 29
# speedup vs baseline: 1.0107x; 1.0107x over previous
"""Trainium2 Bass kernel for nn_DynamicMemoryCell.

Reference computation (per batch element b, N=2048 rows):
    q = x @ Wq.T + bq                  # [N, 128]
    k = h @ Wk.T + bk                  # [N, 128]
    probs = softmax(q @ k.T, axis=-1)  # [N, N]   (output 2)
    u = sigmoid([x, h] @ Wu.T + bu)    # [N, 256]
    r = sigmoid([x, h] @ Wr.T + br)    # [N, 256]
    c = tanh([x, r*h] @ Wc.T + bc)     # [N, 256]
    new_mem = (1-u)*h + u*c            # [N, 256]  (output 1)
(The reference's context_vector / Wv / bv are dead code — not computed here.)

Sharding: data-parallel over batch, one batch element per NeuronCore (B=8).

On-chip layout is feature-major ("transposed"): big tensors live as
[feature (partitions), N (free)] so matmul contractions run over the
partition dim and all biases are per-partition scalars.  The host
pre-transposes inputs / packs weights and transposes new_mem back, so the
device does zero transposes.  Matmul inputs are fp16 (fp16*fp16 products
are exact in fp32 accumulation; input rounding error ~5e-4 relative).
Softmax skips max-subtraction: |scores| <= ~40 so exp() stays in fp32
range, and softmax output is mathematically shift-invariant.

Scheduling: the q/k projections run first, then the 16-query-tile
scores->exp->normalize->store pipeline (memory-bound: 1 MB of probs per
tile).  Gate work is interleaved between score tiles so TensorE/ScalarE
fill the gaps under the DMA-bound probs stream.  Sigmoids are computed as
0.5 + 0.5*tanh(x/2) because tanh lives in the same ScalarE table set as
exp ("exp_and_others") while sigmoid does not — this avoids ~2.7us table
reloads and lets gate activations interleave with the exp stream.  The
q/k bias adds and the 0.5*t+0.5 affines run on VectorE, the new-memory
blend on VectorE, so ScalarE does only tanh/exp.
"""

import numpy as np

B, N = 8, 2048
IN_DIM, MEM_DIM, CTX = 256, 256, 128
NT = N // 512  # 4 free-dim tiles of 512 per [128, N] psum tile
QT = N // 128  # 16 query tiles

_BUILT = None  # cached nc so repeated kernel() calls reuse the program


def _build_nc():
    import concourse.bacc as bacc
    import concourse.tile as tile
    from concourse import mybir

    f16 = mybir.dt.float16
    f32 = mybir.dt.float32
    AF = mybir.ActivationFunctionType
    _MULT = mybir.AluOpType.mult
    _ADD = mybir.AluOpType.add

    # Bacc (not plain Bass): its finalize() runs the lowering passes that
    # split multi-sem waits onto EventSemaphore instructions (TRN2 allows
    # only one sem wait per regular instruction) and insert ACT table loads.
    nc = bacc.Bacc("TRN2", target_bir_lowering=False, debug=False)

    # ---- DRAM parameters (per core) ----
    xT16_d = nc.declare_dram_parameter("xT16", [2, 128, N], f16, isOutput=False)
    hT16_d = nc.declare_dram_parameter("hT16", [2, 128, N], f16, isOutput=False)
    hT32_d = nc.declare_dram_parameter("hT32", [2, 128, N], f32, isOutput=False)
    # weights packed as [128 (contraction-within-chunk), kchunk * out_features]
    wq_d = nc.declare_dram_parameter("wq", [128, 2 * CTX], f16, isOutput=False)
    wk_d = nc.declare_dram_parameter("wk", [128, 2 * CTX], f16, isOutput=False)
    wu_d = nc.declare_dram_parameter("wu", [128, 4 * MEM_DIM], f16, isOutput=False)
    wr_d = nc.declare_dram_parameter("wr", [128, 4 * MEM_DIM], f16, isOutput=False)
    wc_d = nc.declare_dram_parameter("wc", [128, 4 * MEM_DIM], f16, isOutput=False)
    # biases, packed per 128-partition chunk: bq, bk raw; br, bu pre-halved
    # (tanh path: sigmoid(x) = 0.5 + 0.5*tanh(0.5*x + b/2)); bc raw.
    bq_d = nc.declare_dram_parameter("bq", [128, 1], f32, isOutput=False)
    bk_d = nc.declare_dram_parameter("bk", [128, 1], f32, isOutput=False)
    buh_d = nc.declare_dram_parameter("buh", [2, 128, 1], f32, isOutput=False)
    brh_d = nc.declare_dram_parameter("brh", [2, 128, 1], f32, isOutput=False)
    bc_d = nc.declare_dram_parameter("bc", [2, 128, 1], f32, isOutput=False)
    # outputs.  probs leaves the device as fp16: values are normalized
    # softmax outputs in [0,1] (well inside fp16 range) and halving the
    # dominant 16.8 MB/core store is worth the ~5e-4 rounding; the host
    # upcasts to fp32.
    probs_d = nc.declare_dram_parameter("probs", [N, N], f16, isOutput=True)
    nmT_d = nc.declare_dram_parameter("nmT", [2, 128, N], f32, isOutput=True)

    with tile.TileContext(nc) as tc:
        with (
            tc.tile_pool(name="const", bufs=1) as const,
            tc.tile_pool(name="big", bufs=1) as big,
            tc.tile_pool(name="probsp", bufs=5) as probsp,
            tc.tile_pool(name="small", bufs=16) as small,
            tc.tile_pool(name="scr", bufs=2) as scr,
            tc.tile_pool(name="et32p", bufs=3) as et32p,
            tc.tile_pool(name="psum", bufs=2, space="PSUM") as psum,
        ):
            # ---- loads, critical-path first ----
            wq_sb = const.tile([128, 2 * CTX], f16, name="wq_sb")
            wk_sb = const.tile([128, 2 * CTX], f16, name="wk_sb")
            nc.sync.dma_start(out=wq_sb, in_=wq_d[:, :])
            nc.sync.dma_start(out=wk_sb, in_=wk_d[:, :])
            bq_sb = const.tile([128, 1], f32, name="bq_sb")
            bk_sb = const.tile([128, 1], f32, name="bk_sb")
            nc.sync.dma_start(out=bq_sb, in_=bq_d[:, :])
            nc.sync.dma_start(out=bk_sb, in_=bk_d[:, :])

            xt = []
            ht16 = []
            for i in range(2):
                x_i = big.tile([128, N], f16, name=f"xt{i}")
                nc.sync.dma_start(out=x_i, in_=xT16_d[i, :, :])
                xt.append(x_i)
            for i in range(2):
                h_i = big.tile([128, N], f16, name=f"ht16_{i}")
                nc.sync.dma_start(out=h_i, in_=hT16_d[i, :, :])
                ht16.append(h_i)

            wr_sb = const.tile([128, 4 * MEM_DIM], f16, name="wr_sb")
            wu_sb = const.tile([128, 4 * MEM_DIM], f16, name="wu_sb")
            wc_sb = const.tile([128, 4 * MEM_DIM], f16, name="wc_sb")
            nc.sync.dma_start(out=wr_sb, in_=wr_d[:, :])
            nc.sync.dma_start(out=wu_sb, in_=wu_d[:, :])
            nc.sync.dma_start(out=wc_sb, in_=wc_d[:, :])
            brh_sb = const.tile([128, 2], f32, name="brh_sb")
            buh_sb = const.tile([128, 2], f32, name="buh_sb")
            bc_sb = const.tile([128, 2], f32, name="bc_sb")
            for m in range(2):
                nc.sync.dma_start(out=brh_sb[:, m : m + 1], in_=brh_d[m, :, :])
                nc.sync.dma_start(out=buh_sb[:, m : m + 1], in_=buh_d[m, :, :])
                nc.sync.dma_start(out=bc_sb[:, m : m + 1], in_=bc_d[m, :, :])
            ht32 = []
            for i in range(2):
                h32_i = big.tile([128, N], f32, name=f"ht32_{i}")
                nc.sync.dma_start(out=h32_i, in_=hT32_d[i, :, :])
                ht32.append(h32_i)

            combined = xt + ht16  # 4 contraction chunks of the gate matmuls

            # ---- PE warmup ----
            # The PE clock sits at 1.2 GHz until ~3.4us of sustained
            # activity (HAM gate).  Input DMA takes ~6us anyway, so warm the
            # array on a zeroed tile now and the q/k/score matmuls run at
            # 2.4 GHz from the start.
            warm = scr.tile([128, 512], f16, name="warm", tag="warm")
            nc.vector.memset(warm, 0.0)
            wps = psum.tile([128, N], f32, name="wps", tag="ps")
            for _ in range(16):
                nc.tensor.matmul(
                    wps[:, 0:512], lhsT=warm[:, 0:128], rhs=warm,
                    start=True, stop=True,
                )
            # Dummy first ACTIVATE: the ~2.7us exp_and_others table load +
            # drain attaches to the first ScalarE op; hanging it on this
            # no-dependency exp moves it off the q/k -> first-exp critical
            # path to t~7us.
            wact = small.tile([128, 1], f32, name="wact", tag="wact")
            nc.scalar.activation(wact, warm[:, 0:1], AF.Exp)

            # ---- q/k projections (CTX=128 partitions) ----
            # PSUM->SBUF bias+cast: q on VectorE, k on ScalarE Identity —
            # they run in parallel, shortening the path to the first exp.
            q16 = big.tile([128, N], f16, name="q16")
            k16 = big.tile([128, N], f16, name="k16")
            for (w_sb, src, b_sb, dst, via_act) in (
                (wq_sb, xt, bq_sb, q16, False),
                (wk_sb, ht16, bk_sb, k16, True),
            ):
                ps = psum.tile([128, N], f32, name="ps_qk", tag="ps")
                for j in range(NT):
                    js = slice(j * 512, (j + 1) * 512)
                    for k in range(2):
                        nc.tensor.matmul(
                            ps[:, js],
                            lhsT=w_sb[:, k * CTX : (k + 1) * CTX],
                            rhs=src[k][:, js],
                            start=(k == 0),
                            stop=(k == 1),
                        )
                if via_act:
                    nc.scalar.activation(dst, ps, AF.Identity, bias=b_sb)
                else:
                    nc.vector.tensor_scalar_add(dst, ps, b_sb)

            # ---- gate building blocks (emitted interleaved with scores) ----
            def gate_matmuls(w_sb, chunks, m):
                ps = psum.tile([128, N], f32, name="ps_g", tag="ps")
                for j in range(NT):
                    js = slice(j * 512, (j + 1) * 512)
                    for k in range(4):
                        col = k * MEM_DIM + m * 128
                        nc.tensor.matmul(
                            ps[:, js],
                            lhsT=w_sb[:, col : col + 128],
                            rhs=chunks[k][:, js],
                            start=(k == 0),
                            stop=(k == 3),
                        )
                return ps

            r16 = [None, None]
            rh16 = [None, None]
            u16 = [None, None]
            c16 = [None, None]
            p1 = [None, None]  # h*(1-u), precomputed before c is ready

            def r_block(m):
                ps = gate_matmuls(wr_sb, combined, m)
                t = scr.tile([128, N], f16, name="t16", tag="t16")
                nc.scalar.activation(
                    t, ps, AF.Tanh, bias=brh_sb[:, m : m + 1], scale=0.5
                )
                r_m = big.tile([128, N], f16, name=f"r16_{m}")
                nc.vector.tensor_scalar(
                    r_m, t, 0.5, 0.5,
                    op0=_MULT, op1=_ADD,
                )
                rh_m = big.tile([128, N], f16, name=f"rh16_{m}")
                nc.vector.tensor_mul(rh_m, r_m, ht16[m])
                r16[m], rh16[m] = r_m, rh_m

            def u_block(m):
                ps = gate_matmuls(wu_sb, combined, m)
                t = scr.tile([128, N], f16, name="t16", tag="t16")
                nc.scalar.activation(
                    t, ps, AF.Tanh, bias=buh_sb[:, m : m + 1], scale=0.5
                )
                u_m = big.tile([128, N], f16, name=f"u16_{m}")
                nc.vector.tensor_scalar(
                    u_m, t, 0.5, 0.5,
                    op0=_MULT, op1=_ADD,
                )
                u16[m] = u_m
                # 1-u = 0.5 - 0.5*t, then h*(1-u): hides the blend work
                # that doesn't need c, shortening the post-c tail.
                w_m = scr.tile([128, N], f16, name="t16b", tag="t16b")
                nc.vector.tensor_scalar(
                    w_m, t, -0.5, 0.5,
                    op0=_MULT, op1=_ADD,
                )
                p1_m = big.tile([128, N], f32, name=f"p1_{m}")
                nc.vector.tensor_mul(p1_m, w_m, ht32[m])
                p1[m] = p1_m

            def c_block(m):
                ps = gate_matmuls(wc_sb, xt + [rh16[0], rh16[1]], m)
                c_m = big.tile([128, N], f16, name=f"c16_{m}")
                nc.scalar.activation(c_m, ps, AF.Tanh, bias=bc_sb[:, m : m + 1])
                c16[m] = c_m

            def nm_block(m):
                # new_mem = h*(1-u) + u*c, with h*(1-u) precomputed
                t2 = scr.tile([128, N], f16, name="t16c", tag="t16c")
                nc.vector.tensor_mul(t2, u16[m], c16[m])
                nm_m = big.tile([128, N], f32, name=f"nm_{m}")
                nc.vector.tensor_add(nm_m, t2, p1[m])
                nc.sync.dma_start(out=nmT_d[m, :, :], in_=nm_m)

            # ---- one score tile: 128 q rows -> softmax probs -> DRAM ----
            # exp writes fp32 (unnormalized exps overflow fp16), the
            # normalize pass casts to fp16 for the store.
            def score_tile(q):
                ps = psum.tile([128, N], f32, name="ps_s", tag="ps")
                for j in range(NT):
                    js = slice(j * 512, (j + 1) * 512)
                    nc.tensor.matmul(
                        ps[:, js],
                        lhsT=q16[:, q * 128 : (q + 1) * 128],
                        rhs=k16[:, js],
                        start=True,
                        stop=True,
                    )
                et = et32p.tile([128, N], f32, name="et", tag="et")
                ssum = small.tile([128, 1], f32, name="ssum", tag="ssum")
                nc.scalar.activation(et, ps, AF.Exp, accum_out=ssum)
                rsum = small.tile([128, 1], f32, name="rsum", tag="rsum")
                nc.vector.reciprocal(rsum, ssum)
                etp = probsp.tile([128, N], f16, name="etp", tag="etp")
                nc.vector.tensor_scalar_mul(etp, et, rsum)
                nc.sync.dma_start(out=probs_d[q * 128 : (q + 1) * 128, :], in_=etp)

            # ---- interleaved schedule: gate work shares the two PSUM
            # slots with the score tiles, filling TensorE/ScalarE gaps in
            # the DMA-bound probs pipeline ----
            gate_work = {
                1: lambda: r_block(0),
                3: lambda: r_block(1),
                5: lambda: u_block(0),
                7: lambda: u_block(1),
                9: lambda: c_block(0),
                11: lambda: c_block(1),
                12: lambda: nm_block(0),
                13: lambda: nm_block(1),
            }
            for q in range(QT):
                score_tile(q)
                w = gate_work.get(q)
                if w is not None:
                    w()

    nc.finalize()
    return nc


def _pack_qk(w):
    # Wq/Wk [CTX=128, D=256] -> lhsT chunks [128 (d within chunk), 2*128]
    return np.ascontiguousarray(
        w.T.reshape(2, 128, CTX).transpose(1, 0, 2).reshape(128, 2 * CTX)
    ).astype(np.float16)


def _pack_gate(w):
    # Wu/Wr/Wc [M=256, D=512] -> lhsT chunks [128 (d within chunk), 4*256]
    return np.ascontiguousarray(
        w.T.reshape(4, 128, MEM_DIM).transpose(1, 0, 2).reshape(128, 4 * MEM_DIM)
    ).astype(np.float16)


def _in_maps(inputs):
    x = np.asarray(inputs["input_tensor"], np.float32)   # [B, N, 256]
    h = np.asarray(inputs["previous_memory"], np.float32)
    shared = {
        "wq": _pack_qk(np.asarray(inputs["Wq"], np.float32)),
        "wk": _pack_qk(np.asarray(inputs["Wk"], np.float32)),
        "wu": _pack_gate(np.asarray(inputs["Wu"], np.float32)),
        "wr": _pack_gate(np.asarray(inputs["Wr"], np.float32)),
        "wc": _pack_gate(np.asarray(inputs["Wc"], np.float32)),
        "bq": np.asarray(inputs["bq"], np.float32).reshape(128, 1),
        "bk": np.asarray(inputs["bk"], np.float32).reshape(128, 1),
        "buh": np.asarray(inputs["bu"], np.float32).reshape(2, 128, 1) * 0.5,
        "brh": np.asarray(inputs["br"], np.float32).reshape(2, 128, 1) * 0.5,
        "bc": np.asarray(inputs["bc"], np.float32).reshape(2, 128, 1),
    }
    in_maps = []
    for b in range(B):
        xT = np.ascontiguousarray(x[b].T)  # [256, N]
        hT = np.ascontiguousarray(h[b].T)
        m = dict(shared)
        m["xT16"] = xT.reshape(2, 128, N).astype(np.float16)
        m["hT16"] = hT.reshape(2, 128, N).astype(np.float16)
        m["hT32"] = hT.reshape(2, 128, N)
        in_maps.append(m)
    return in_maps


def kernel(**inputs):
    from concourse.bass_utils import run_bass_kernel_spmd

    global _BUILT
    if _BUILT is None:
        _BUILT = _build_nc()
    nc = _BUILT

    res = run_bass_kernel_spmd(nc, _in_maps(inputs), list(range(B))).results

    probs = np.stack([res[b]["probs"] for b in range(B)]).astype(np.float32)
    new_mem = np.stack(
        [res[b]["nmT"].reshape(MEM_DIM, N).T for b in range(B)]
    )
    return np.ascontiguousarray(new_mem), probs
